# revision 1
# baseline (speedup 1.0000x reference)
"""Segment-mean GNN message passing (scatter-mean) on 8 TRN2 NeuronCores.

out[d] = mean over edges e with col[e]==d of x[row[e]]   (empty segments -> 0)

Design (1D graph partition per the sharding hint):
- Destinations sharded across 8 cores (6250 each); edges partitioned by
  destination on host; each destination's edge list padded to groups of G=3
  slots, each chunk (128 dests) padded to a core-shared group-tile count
  (SPMD: one instruction stream). Remote x rows are materialized host-side
  per slot (halo exchange) and streamed to SBUF in bf16, group members
  adjacent in the free dimension.
- VectorE folds slots 0+1 of every group with one chunk-batched add; it also
  builds the per-group one-hot scatter matrices (is_equal of group dest ids
  against an iota row) and applies 1/degree (degree = graph preprocessing).
- TensorE scatters group sums into the 128-dest chunk accumulator in PSUM
  (two matmuls per 128-group block: folded pair + third slot, same one-hot
  weights, f32 accumulation), overlapped with the sliced xg upload via
  per-slice semaphores.
"""

import sys

for _p in ("/opt/trn_rl_repo",):
    if _p not in sys.path:
        sys.path.insert(0, _p)

import numpy as np
import ml_dtypes

N_NODES = 50000
D_FEAT = 64
N_EDGES = 800000
NCORES = 8
SPAN = N_NODES // NCORES  # 6250 dests per core
P = 128
NCHUNK = (SPAN + P - 1) // P  # 49 (last chunk has 106 dests)
G = 3  # slots per group
QPT = P // G  # quads per level-1 tile (32)
PS2_BUFS = 4
QS_BUFS = 4
N_SLICES = 8


def _preprocess(x, edge_index):
    x = np.ascontiguousarray(x, dtype=np.float32)
    row = edge_index[0].astype(np.int64)
    col = edge_index[1].astype(np.int64)

    deg = np.bincount(col, minlength=N_NODES).astype(np.float32)
    recip_full = (1.0 / np.maximum(deg, 1.0)).astype(np.float32)

    core = col // SPAN
    lcol = col - core * SPAN
    chunk = lcol // P

    # quads needed per (core, chunk): sum over its dests of ceil(deg/4)
    qneed = np.zeros((NCORES, NCHUNK), np.int64)
    dq = -(-deg.astype(np.int64) // G)  # ceil(deg/G) per dest, 0 for empty
    dcore = np.arange(N_NODES) // SPAN
    dchunk = (np.arange(N_NODES) - dcore * SPAN) // P
    np.add.at(qneed, (dcore, dchunk), dq)
    T2 = np.maximum(1, -(-qneed.max(axis=0) // P)).astype(np.int64)  # [NCHUNK]
    S2 = np.zeros(NCHUNK + 1, np.int64)
    S2[1:] = np.cumsum(T2)
    tt2 = int(S2[NCHUNK])  # total level-2 tiles' quad-blocks
    tt1 = tt2 * G  # level-1 slot tiles
    e_total = tt1 * P

    bounds = [round(s * tt2 / N_SLICES) for s in range(N_SLICES + 1)]
    slices = [(bounds[s], bounds[s + 1]) for s in range(N_SLICES)]

    cfg = dict(T2=T2, S2=S2, tt1=tt1, tt2=tt2, slices=slices)

    iota = np.broadcast_to(np.arange(P, dtype=ml_dtypes.bfloat16), (P, P)).copy()

    in_maps = []
    for ci in range(NCORES):
        m = core == ci
        r_i, ch_i, l_i = row[m], chunk[m], lcol[m]
        d_i = l_i - ch_i * P  # dest within chunk [0,128)
        order = np.lexsort((r_i, d_i, ch_i))
        r_i, ch_i, d_i = r_i[order], ch_i[order], d_i[order]

        # per-edge slot: edges of dest d sit in quads; dest quad ranges are
        # laid out consecutively within the chunk's quad span.
        ldest = ch_i * P + d_i  # local dest id 0..6271
        equad = np.zeros(NCHUNK * P, np.int64)
        equad[:SPAN] = dq[ci * SPAN : (ci + 1) * SPAN]
        # quad start per local dest within its chunk
        qstart = np.zeros(NCHUNK * P, np.int64)
        for c in range(NCHUNK):
            a, b = c * P, (c + 1) * P
            qs = np.zeros(P, np.int64)
            qs[1:] = np.cumsum(equad[a : b - 1])
            qstart[a:b] = S2[c] * P + qs
        # position of edge within its dest
        first = np.zeros(len(r_i), bool)
        first[0] = True
        first[1:] = ldest[1:] != ldest[:-1]
        gidx = np.arange(len(r_i))
        dstart = np.zeros(len(r_i), np.int64)
        dstart[first] = gidx[first]
        dstart = np.maximum.accumulate(dstart)
        pos = gidx - dstart  # edge position within its dest
        slot = qstart[ldest] * G + pos

        xg = np.zeros((e_total, D_FEAT), np.float32)
        xg[slot] = x[r_i]
        xg_pm = np.ascontiguousarray(
            xg.reshape(tt2, P, G, D_FEAT).transpose(1, 0, 2, 3).astype(
                ml_dtypes.bfloat16
            )
        )  # [128, tt2, G, 64]: pair partners adjacent in free dim

        # quad -> dest-within-chunk (or -1 for pad quads)
        colq = np.full(tt2 * P, -1.0, np.float32)
        for c in range(NCHUNK):
            a, b = c * P, (c + 1) * P
            nq = equad[a:b]
            colq[np.repeat(qstart[a:b], nq) + _ragged_arange(nq)] = np.repeat(
                np.arange(P), nq
            )
        colq_pm = np.ascontiguousarray(colq.reshape(tt2, P).T)  # [128, tt2]

        rc = np.zeros(NCHUNK * P, np.float32)
        rc[:SPAN] = recip_full[ci * SPAN : (ci + 1) * SPAN]
        recip = np.ascontiguousarray(rc.reshape(NCHUNK, P).T)

        in_maps.append(
            {
                "xg": xg_pm,
                "colq": colq_pm,
                "recip": recip,
                "iota": iota,
            }
        )
    return cfg, in_maps


def _ragged_arange(counts):
    """[0..c0), [0..c1), ... concatenated."""
    total = int(counts.sum())
    out = np.arange(total)
    starts = np.zeros(len(counts), np.int64)
    starts[1:] = np.cumsum(counts)[:-1]
    out -= np.repeat(starts, counts)
    return out


def _build(cfg):
    import concourse.bacc as bacc
    import concourse.mybir as mybir

    T2, S2 = cfg["T2"], cfg["S2"]
    tt1, tt2, slices = cfg["tt1"], cfg["tt2"], cfg["slices"]
    t2max = int(T2.max())

    slice_of_blk = np.zeros(tt2, np.int64)
    for s, (b0, b1) in enumerate(slices):
        slice_of_blk[b0:b1] = s

    nc = bacc.Bacc()
    f32 = mybir.dt.float32
    bf16 = mybir.dt.bfloat16
    xg_ext = nc.declare_dram_parameter("xg", [P, tt2, G, D_FEAT], bf16, isOutput=False)
    colq_ext = nc.declare_dram_parameter("colq", [P, tt2], f32, isOutput=False)
    recip_ext = nc.declare_dram_parameter("recip", [P, NCHUNK], f32, isOutput=False)
    iota_ext = nc.declare_dram_parameter("iota", [P, P], bf16, isOutput=False)
    out_ext = nc.declare_dram_parameter("out", [SPAN, D_FEAT], f32, isOutput=True)

    colq_sb = nc.alloc_sbuf_tensor("colq_sb", [P, tt2], f32)
    recip_sb = nc.alloc_sbuf_tensor("recip_sb", [P, NCHUNK], f32)
    iota_sb = nc.alloc_sbuf_tensor("iota_sb", [P, P], bf16)
    xg = nc.alloc_sbuf_tensor("xg_sb", [P, tt2, G, D_FEAT], bf16)
    qsum = nc.alloc_sbuf_tensor("qsum", [P, tt2, D_FEAT], bf16)
    oh2 = nc.alloc_sbuf_tensor("oh2", [P, 2, t2max, P], bf16)
    outst = nc.alloc_sbuf_tensor("outst", [P, NCHUNK, D_FEAT], f32)
    ps2 = nc.alloc_psum_tensor("ps2", [P, PS2_BUFS, 512], f32)

    # level-2 block index -> (chunk, k-within-chunk)
    chunk_of_b2 = np.searchsorted(S2[1:], np.arange(tt2), side="right")

    with (
        nc.Block() as block,
        nc.semaphore("sem_in") as sem_in,
        nc.semaphore("sem_x0") as sem_x0,
        nc.semaphore("sem_x1") as sem_x1,
        nc.semaphore("sem_x2") as sem_x2,
        nc.semaphore("sem_x3") as sem_x3,
        nc.semaphore("sem_x4") as sem_x4,
        nc.semaphore("sem_x5") as sem_x5,
        nc.semaphore("sem_x6") as sem_x6,
        nc.semaphore("sem_x7") as sem_x7,
        nc.semaphore("sem_oh") as sem_oh,
        nc.semaphore("sem_ps") as sem_ps,
        nc.semaphore("sem_l2") as sem_l2,
        nc.semaphore("sem_div") as sem_div,
        nc.semaphore("sem_out") as sem_out,
    ):
        sem_x = [sem_x0, sem_x1, sem_x2, sem_x3, sem_x4, sem_x5, sem_x6, sem_x7]

        @block.sync
        def _(sync):
            sync.dma_start(out=colq_sb[:], in_=colq_ext[:]).then_inc(sem_in, 16)
            sync.dma_start(out=iota_sb[:], in_=iota_ext[:]).then_inc(sem_in, 16)
            sync.dma_start(out=recip_sb[:], in_=recip_ext[:]).then_inc(sem_in, 16)
            for s, (b0, b1) in enumerate(slices):
                sync.dma_start(
                    out=xg[:, b0:b1, :], in_=xg_ext[:, b0:b1, :]
                ).then_inc(sem_x[s], 16)

        @block.vector
        def _(vector):
            vector.wait_ge(sem_in, 48)

            last_s = -1
            for c in range(NCHUNK):
                if c >= 2:
                    vector.wait_ge(sem_l2, int(S2[c - 1]))  # oh2 buf c%2 free
                s_end = int(slice_of_blk[int(S2[c + 1]) - 1])
                while last_s < s_end:
                    last_s += 1
                    vector.wait_ge(sem_x[last_s], 16)
                    b0, b1 = slices[last_s]
                    vector.tensor_tensor(
                        out=qsum[:, b0:b1, :],
                        in0=xg[:, b0:b1, 0, :],
                        in1=xg[:, b0:b1, 1, :],
                        op=mybir.AluOpType.add,
                    ).then_inc(sem_ps, 1)
                for k in range(int(T2[c])):
                    vector.tensor_scalar(
                        out=oh2[:, c % 2, k, :],
                        in0=iota_sb[:],
                        scalar1=colq_sb[:, int(S2[c]) + k : int(S2[c]) + k + 1],
                        scalar2=None,
                        op0=mybir.AluOpType.is_equal,
                    ).then_inc(sem_oh, 1)


        @block.scalar
        def _(act):
            act.wait_ge(sem_in, 48)
            for c in range(NCHUNK):
                act.wait_ge(sem_l2, int(S2[c]) + int(T2[c]))
                act.activation(
                    out=outst[:, c, :],
                    in_=ps2[:, c % PS2_BUFS, 0:D_FEAT],
                    func=mybir.ActivationFunctionType.Copy,
                    scale=recip_sb[:, c : c + 1],
                ).then_inc(sem_div, 1)

        @block.tensor
        def _(pe):
            for b2 in range(tt2):
                c = int(chunk_of_b2[b2])
                k = b2 - int(S2[c])
                if k == 0 and c >= PS2_BUFS:
                    pe.wait_ge(sem_div, c - (PS2_BUFS - 1))
                if k == 0:
                    pe.wait_ge(sem_oh, int(S2[c]) + int(T2[c]))
                    pe.wait_ge(sem_ps, int(slice_of_blk[int(S2[c + 1]) - 1]) + 1)
                pe.matmul(
                    ps2[:, c % PS2_BUFS, 0:D_FEAT],
                    lhsT=oh2[:, c % 2, k, :],
                    rhs=qsum[:, b2, :],
                    start=(k == 0),
                    stop=False,
                )
                pe.matmul(
                    ps2[:, c % PS2_BUFS, 0:D_FEAT],
                    lhsT=oh2[:, c % 2, k, :],
                    rhs=xg[:, b2, 2, :],
                    start=False,
                    stop=(k == int(T2[c]) - 1),
                ).then_inc(sem_l2, 1)

        @block.sync
        def _(sync):
            sync.wait_ge(sem_div, NCHUNK)
            full = (NCHUNK - 1) * P
            sync.dma_start(
                out=out_ext[0:full, :].rearrange("(c p) f -> p c f", p=P),
                in_=outst[:, 0 : NCHUNK - 1, :],
            ).then_inc(sem_out, 16)
            sync.dma_start(
                out=out_ext[full:SPAN, :],
                in_=outst[0 : SPAN - full, NCHUNK - 1, :],
            ).then_inc(sem_out, 16)
            sync.wait_ge(sem_out, 32)

    nc.finalize()
    return nc


def _get_built(x, edge_index):
    cfg, in_maps = _preprocess(x, edge_index)
    nc = _build(cfg)
    return cfg, in_maps, nc


def kernel(x, edge_index):
    from concourse.bass_utils import run_bass_kernel_spmd

    cfg, in_maps, nc = _get_built(np.asarray(x), np.asarray(edge_index))
    res = run_bass_kernel_spmd(nc, in_maps, core_ids=list(range(NCORES)))
    out = np.concatenate([res.results[i]["out"] for i in range(NCORES)], axis=0)
    return out.astype(np.float32)



# revision 2
# speedup vs baseline: 1.8196x; 1.8196x over previous
"""Segment-mean GNN message passing (scatter-mean) on 8 TRN2 NeuronCores.

out[d] = mean over edges e with col[e]==d of x[row[e]]   (empty segments -> 0)

Design (1D graph partition by destination, per the sharding hint):
- Destinations sharded across 8 cores (6250 each). Per core, local dests are
  sorted by degree and grouped into 49 chunks of 128 (rank r -> chunk r//128,
  partition r%128). Because chunk degree profiles are nearly identical across
  cores, one shared block count T2[c] (cross-core max deg in the chunk) gives
  a single SPMD instruction stream.
- Edge k of dest p in chunk c occupies slot [p, S2[c]+k] of a dense fp8-e3m4
  feature stream xg [128, B_tot, 64] materialized host-side (halo exchange on
  host); empty slots are zero. Scatter-add then degenerates to accumulating
  consecutive blocks: PE matmuls with a constant fp8 identity lhsT sum blocks
  S2[c]..S2[c]+T2[c]-1 into one PSUM bank per chunk (f32 accumulation) - no
  per-block one-hot construction at all.
- ACT copies each finished bank to SBUF as bf16 scaled by 1/deg (f32), and the
  output streams back in 8 grouped DMAs as [128, 49, 64] bf16 which the host
  unpermutes/casts. fp8-e3m4 payload (4 mantissa bits) keeps max rel err
  ~1.3e-2 on randn features; inbound DMA drops 14.45MB -> 6.6MB per core.
"""

import sys

for _p in ("/opt/trn_rl_repo",):
    if _p not in sys.path:
        sys.path.insert(0, _p)

import numpy as np
import ml_dtypes

N_NODES = 50000
D_FEAT = 64
N_EDGES = 800000
NCORES = 8
SPAN = N_NODES // NCORES  # 6250 dests per core
P = 128
NCHUNK = (SPAN + P - 1) // P  # 49 chunks (6272 ranks, 22 pads)
NPAD = NCHUNK * P - SPAN
N_SLICES = 10
N_OUTG = 8
PS_BUFS = 8
FP8 = ml_dtypes.float8_e3m4


def _preprocess(x, edge_index):
    x = np.ascontiguousarray(x, dtype=np.float32)
    row = edge_index[0].astype(np.int64)
    col = edge_index[1].astype(np.int64)

    deg = np.bincount(col, minlength=N_NODES).astype(np.int64)
    xq = x.astype(FP8)  # one global fp8 cast of the node features

    # per-core degree-sorted rank layout (pads first, ascending degree)
    orders = []
    chunk_max = np.zeros((NCORES, NCHUNK), np.int64)
    for ci in range(NCORES):
        d = deg[ci * SPAN : (ci + 1) * SPAN]
        order = np.argsort(d, kind="stable")  # local dest ids, ascending deg
        orders.append(order)
        ds = np.concatenate([np.zeros(NPAD, np.int64), d[order]])
        chunk_max[ci] = ds.reshape(NCHUNK, P).max(axis=1)
    T2 = np.maximum(1, chunk_max.max(axis=0))  # [NCHUNK] shared across cores

    # stream order: big chunks first so the pipeline tail is short
    stream = np.argsort(-T2, kind="stable")  # stream pos j -> chunk id
    T2s = T2[stream]  # blocks per stream pos
    S2 = np.zeros(NCHUNK + 1, np.int64)
    S2[1:] = np.cumsum(T2s)
    B_tot = int(S2[NCHUNK])

    # chunk-aligned DMA slices of ~equal block count
    cuts = [0]
    for s in range(1, N_SLICES):
        tgt = s * B_tot / N_SLICES
        cuts.append(int(np.argmin(np.abs(S2[1:NCHUNK] - tgt)) + 1))
    cuts.append(NCHUNK)
    cuts = sorted(set(cuts))
    slices = [(cuts[i], cuts[i + 1]) for i in range(len(cuts) - 1)]  # chunk idx

    # output DMA groups (chunk-aligned, ~equal)
    og = [round(g * NCHUNK / N_OUTG) for g in range(N_OUTG + 1)]
    outg = [(og[g], og[g + 1]) for g in range(N_OUTG)]

    streampos = np.empty(NCHUNK, np.int64)
    streampos[stream] = np.arange(NCHUNK)

    cfg = dict(T2s=T2s, S2=S2, B_tot=B_tot, slices=slices, outg=outg,
               stream=stream, orders=orders)

    ident = np.eye(P, dtype=FP8)

    in_maps = []
    for ci in range(NCORES):
        order = orders[ci]
        rank_of_dest = np.empty(SPAN, np.int64)
        rank_of_dest[order] = np.arange(SPAN) + NPAD

        m = (col >= ci * SPAN) & (col < (ci + 1) * SPAN)
        r_e, c_e = row[m], col[m] - ci * SPAN
        rk = rank_of_dest[c_e]
        es = np.argsort(rk, kind="stable")
        r_e, rk = r_e[es], rk[es]
        # edge position within its dest
        first = np.ones(len(rk), bool)
        first[1:] = rk[1:] != rk[:-1]
        gidx = np.arange(len(rk))
        dstart = np.where(first, gidx, 0)
        dstart = np.maximum.accumulate(dstart)
        pos = gidx - dstart

        p_e = rk % P
        blk = S2[streampos[rk // P]] + pos

        xg = np.zeros((P, B_tot, D_FEAT), FP8)
        xg[p_e, blk] = xq[r_e]

        dd = np.concatenate([np.zeros(NPAD, np.int64), deg[ci * SPAN:(ci + 1) * SPAN][order]])
        recip = np.zeros((P, NCHUNK), np.float32)
        rr = (1.0 / np.maximum(dd, 1)).astype(np.float32) * (dd > 0)
        recip[:, streampos] = rr.reshape(NCHUNK, P).T[:, :]

        in_maps.append({"xg": xg, "recip": recip, "ident": ident})
    return cfg, in_maps


def _build(cfg):
    import concourse.bacc as bacc
    import concourse.mybir as mybir

    T2s, S2, B_tot = cfg["T2s"], cfg["S2"], cfg["B_tot"]
    slices, outg = cfg["slices"], cfg["outg"]
    nsl = len(slices)

    nc = bacc.Bacc()
    f32 = mybir.dt.float32
    bf16 = mybir.dt.bfloat16
    fp8 = mybir.dt.float8e3
    xg_ext = nc.declare_dram_parameter("xg", [P, B_tot, D_FEAT], fp8, isOutput=False)
    recip_ext = nc.declare_dram_parameter("recip", [P, NCHUNK], f32, isOutput=False)
    ident_ext = nc.declare_dram_parameter("ident", [P, P], fp8, isOutput=False)
    out_ext = nc.declare_dram_parameter("out", [P, NCHUNK, D_FEAT], bf16, isOutput=True)

    recip_sb = nc.alloc_sbuf_tensor("recip_sb", [P, NCHUNK], f32)
    ident_sb = nc.alloc_sbuf_tensor("ident_sb", [P, P], fp8)
    xg = nc.alloc_sbuf_tensor("xg_sb", [P, B_tot, D_FEAT], fp8)
    outst = nc.alloc_sbuf_tensor("outst", [P, NCHUNK, D_FEAT], bf16)
    ps = nc.alloc_psum_tensor("ps", [P, PS_BUFS, 512], f32)

    # stream chunk j -> slice index
    slice_of_chunk = np.zeros(NCHUNK, np.int64)
    for s, (c0, c1) in enumerate(slices):
        slice_of_chunk[c0:c1] = s

    with (
        nc.Block() as block,
        nc.semaphore("sem_in") as sem_in,
        nc.semaphore("sem_x0") as sem_x0,
        nc.semaphore("sem_x1") as sem_x1,
        nc.semaphore("sem_x2") as sem_x2,
        nc.semaphore("sem_x3") as sem_x3,
        nc.semaphore("sem_x4") as sem_x4,
        nc.semaphore("sem_x5") as sem_x5,
        nc.semaphore("sem_x6") as sem_x6,
        nc.semaphore("sem_x7") as sem_x7,
        nc.semaphore("sem_x8") as sem_x8,
        nc.semaphore("sem_x9") as sem_x9,
        nc.semaphore("sem_pe") as sem_pe,
        nc.semaphore("sem_div") as sem_div,
        nc.semaphore("sem_out") as sem_out,
    ):
        sem_x = [sem_x0, sem_x1, sem_x2, sem_x3, sem_x4, sem_x5, sem_x6,
                 sem_x7, sem_x8, sem_x9][:nsl]

        @block.sync
        def _(sync):
            sync.dma_start(out=ident_sb[:], in_=ident_ext[:]).then_inc(sem_in, 16)
            sync.dma_start(out=recip_sb[:], in_=recip_ext[:]).then_inc(sem_in, 16)
            for s, (c0, c1) in enumerate(slices):
                b0, b1 = int(S2[c0]), int(S2[c1])
                sync.dma_start(
                    out=xg[:, b0:b1, :], in_=xg_ext[:, b0:b1, :]
                ).then_inc(sem_x[s], 16)
            for g, (c0, c1) in enumerate(outg):
                sync.wait_ge(sem_div, c1)
                sync.dma_start(
                    out=out_ext[:, c0:c1, :], in_=outst[:, c0:c1, :]
                ).then_inc(sem_out, 16)
            sync.wait_ge(sem_out, 16 * len(outg))

        @block.tensor
        def _(pe):
            pe.wait_ge(sem_in, 32)
            last_s = -1
            for j in range(NCHUNK):
                s = int(slice_of_chunk[j])
                if s > last_s:
                    pe.wait_ge(sem_x[s], 16)
                    last_s = s
                if j >= PS_BUFS:
                    pe.wait_ge(sem_div, j - (PS_BUFS - 1))
                t2 = int(T2s[j])
                for k in range(t2):
                    mm = pe.matmul(
                        ps[:, j % PS_BUFS, 0:D_FEAT],
                        lhsT=ident_sb[:],
                        rhs=xg[:, int(S2[j]) + k, :],
                        start=(k == 0),
                        stop=(k == t2 - 1),
                    )
                    if k == t2 - 1:
                        mm.then_inc(sem_pe, 1)

        @block.scalar
        def _(act):
            act.wait_ge(sem_in, 32)
            for j in range(NCHUNK):
                act.wait_ge(sem_pe, j + 1)
                act.activation(
                    out=outst[:, j, :],
                    in_=ps[:, j % PS_BUFS, 0:D_FEAT],
                    func=mybir.ActivationFunctionType.Copy,
                    scale=recip_sb[:, j : j + 1],
                ).then_inc(sem_div, 1)

    nc.finalize()
    return nc


def _get_built(x, edge_index):
    cfg, in_maps = _preprocess(x, edge_index)
    nc = _build(cfg)
    return cfg, in_maps, nc


def _postprocess(cfg, outs):
    """outs: list per core of [P, NCHUNK, D_FEAT] arrays -> [N_NODES, D] f32."""
    stream, orders = cfg["stream"], cfg["orders"]
    full = np.empty((N_NODES, D_FEAT), np.float32)
    rank_src = (stream[:, None] * P + np.arange(P)[None, :]).ravel()
    for ci in range(NCORES):
        o = np.asarray(outs[ci]).astype(np.float32)  # [P, NCHUNK, D]
        by_rank = np.empty((NCHUNK * P, D_FEAT), np.float32)
        by_rank[rank_src] = o.transpose(1, 0, 2).reshape(-1, D_FEAT)
        full[ci * SPAN + orders[ci]] = by_rank[NPAD:]
    return full


def kernel(x, edge_index):
    from concourse.bass_utils import run_bass_kernel_spmd

    cfg, in_maps, nc = _get_built(np.asarray(x), np.asarray(edge_index))
    res = run_bass_kernel_spmd(nc, in_maps, core_ids=list(range(NCORES)))
    return _postprocess(cfg, [res.results[i]["out"] for i in range(NCORES)])


# revision 5
# speedup vs baseline: 2.2391x; 1.2306x over previous
"""Segment-mean GNN message passing (scatter-mean) on 8 TRN2 NeuronCores.

out[d] = mean over edges e with col[e]==d of x[row[e]]   (empty segments -> 0)

Design (1D graph partition by destination, per the sharding hint):
- Destinations sharded across 8 cores (6250 each). Per core, local dests are
  sorted by degree and grouped into 49 chunks of 128 (rank r -> chunk r//128,
  partition r%128). Because chunk degree profiles are nearly identical across
  cores, one shared block count T2[c] (cross-core max deg in the chunk) gives
  a single SPMD instruction stream.
- Edge k of dest p in chunk c occupies slot [p, S2[c]+k] of a dense fp8-e3m4
  feature stream xg [128, B_tot, 64] materialized host-side (halo exchange on
  host); empty slots are zero. Scatter-add then degenerates to summing
  consecutive blocks - no per-block one-hot construction at all. PE matmuls
  with a constant fp8 identity lhsT accumulate most chunks into PSUM (f32);
  DVE tensor_reduce handles a balanced subset of chunks in parallel.
- ACT copies each finished chunk to SBUF as bf16 scaled by 1/deg (f32).
- DMA issue cost (~1.7us of issuing-engine sequencer per dma_start) is spread
  across engines: xg slices on SP, ident on DVE, recip on ACT, grouped output
  DMAs on the otherwise idle Pool/GpSimd engine. The host unpermutes/casts the
  [128, 49, 64] bf16 result. fp8-e3m4 (4 mantissa bits) keeps max rel err
  ~1.3e-2 on randn features; inbound DMA is 6.6MB/core (vs 14.45MB baseline).
"""

import sys

for _p in ("/opt/trn_rl_repo",):
    if _p not in sys.path:
        sys.path.insert(0, _p)

import numpy as np
import ml_dtypes

N_NODES = 50000
D_FEAT = 64
N_EDGES = 800000
NCORES = 8
SPAN = N_NODES // NCORES  # 6250 dests per core
P = 128
NCHUNK = (SPAN + P - 1) // P  # 49 chunks (6272 ranks, 22 pads)
NPAD = NCHUNK * P - SPAN
PS_BUFS = 8
DVE_TMPS = 4
FP8 = ml_dtypes.float8_e3m4

# engine-time model used for static PE/DVE load balancing (ns)
R_PE = 27.0  # per block on PE (64 rows @ 2.4GHz + overhead)
R_DVE = 67.0  # per block on DVE (64 elems/lane @ 0.96GHz, fp8 1x)
F_DVE = 170.0  # fixed per DVE reduce instruction


def _preprocess(x, edge_index):
    x = np.ascontiguousarray(x, dtype=np.float32)
    row = edge_index[0].astype(np.int64)
    col = edge_index[1].astype(np.int64)

    deg = np.bincount(col, minlength=N_NODES).astype(np.int64)
    xq = x.astype(FP8)  # one global fp8 cast of the node features

    # per-core degree-sorted rank layout (pads first, ascending degree)
    orders = []
    chunk_max = np.zeros((NCORES, NCHUNK), np.int64)
    for ci in range(NCORES):
        d = deg[ci * SPAN : (ci + 1) * SPAN]
        order = np.argsort(d, kind="stable")  # local dest ids, ascending deg
        orders.append(order)
        ds = np.concatenate([np.zeros(NPAD, np.int64), d[order]])
        chunk_max[ci] = ds.reshape(NCHUNK, P).max(axis=1)
    T2 = np.maximum(1, chunk_max.max(axis=0))  # [NCHUNK] shared across cores

    # stream order: a small starter chunk first (fast pipeline fill), then
    # descending T2 so the smallest chunks drain last (short tail)
    desc = list(np.argsort(-T2, kind="stable"))
    starter = min(desc, key=lambda c: abs(int(T2[c]) - 12))
    stream = np.array([starter] + [c for c in desc if c != starter], np.int64)
    T2s = T2[stream]
    S2 = np.zeros(NCHUNK + 1, np.int64)
    S2[1:] = np.cumsum(T2s)
    B_tot = int(S2[NCHUNK])

    # chunk-aligned DMA slices: starter alone, then ~equal, tapering at end
    weights = [1.0] * 6 + [0.75, 0.6, 0.45]
    rest = B_tot - int(T2s[0])
    tot_w = sum(weights)
    targets = []
    acc = int(T2s[0])
    for w in weights[:-1]:
        acc += rest * w / tot_w
        targets.append(acc)
    cuts = [0, 1]
    for tgt in targets:
        c = int(np.argmin(np.abs(S2[1:NCHUNK] - tgt)) + 1)
        cuts.append(c)
    cuts.append(NCHUNK)
    cuts = sorted(set(cuts))
    slices = [(cuts[i], cuts[i + 1]) for i in range(len(cuts) - 1)]  # chunk idx

    # greedy static PE/DVE split per chunk (balanced finish times)
    eng = np.zeros(NCHUNK, np.int64)  # 0=PE, 1=DVE
    t_pe = t_dve = 0.0
    for j in range(NCHUNK):
        b = float(T2s[j])
        if t_pe + R_PE * b <= t_dve + R_DVE * b + F_DVE:
            eng[j] = 0
            t_pe += R_PE * b
        else:
            eng[j] = 1
            t_dve += R_DVE * b + F_DVE

    # output DMA groups (chunk-aligned), tapering sizes so the tail is short
    og = [0, 16, 29, 39, 45, NCHUNK]
    outg = [(og[g], og[g + 1]) for g in range(len(og) - 1)]

    streampos = np.empty(NCHUNK, np.int64)
    streampos[stream] = np.arange(NCHUNK)

    cfg = dict(T2s=T2s, S2=S2, B_tot=B_tot, slices=slices, outg=outg,
               stream=stream, orders=orders, eng=eng)

    ident = np.eye(P, dtype=FP8)

    in_maps = []
    for ci in range(NCORES):
        order = orders[ci]
        rank_of_dest = np.empty(SPAN, np.int64)
        rank_of_dest[order] = np.arange(SPAN) + NPAD

        m = (col >= ci * SPAN) & (col < (ci + 1) * SPAN)
        r_e, c_e = row[m], col[m] - ci * SPAN
        rk = rank_of_dest[c_e]
        es = np.argsort(rk, kind="stable")
        r_e, rk = r_e[es], rk[es]
        # edge position within its dest
        first = np.ones(len(rk), bool)
        first[1:] = rk[1:] != rk[:-1]
        gidx = np.arange(len(rk))
        dstart = np.where(first, gidx, 0)
        dstart = np.maximum.accumulate(dstart)
        pos = gidx - dstart

        p_e = rk % P
        blk = S2[streampos[rk // P]] + pos

        xg = np.zeros((P, B_tot, D_FEAT), FP8)
        xg[p_e, blk] = xq[r_e]

        dd = np.concatenate(
            [np.zeros(NPAD, np.int64), deg[ci * SPAN:(ci + 1) * SPAN][order]]
        )
        recip = np.zeros((P, NCHUNK), np.float32)
        rr = (1.0 / np.maximum(dd, 1)).astype(np.float32) * (dd > 0)
        recip[:, streampos] = rr.reshape(NCHUNK, P).T[:, :]

        in_maps.append({"xg": xg, "recip": recip, "ident": ident})
    return cfg, in_maps


def _build(cfg):
    import concourse.bacc as bacc
    import concourse.mybir as mybir

    T2s, S2, B_tot = cfg["T2s"], cfg["S2"], cfg["B_tot"]
    slices, outg, eng = cfg["slices"], cfg["outg"], cfg["eng"]
    nsl = len(slices)
    assert nsl <= 10

    nc = bacc.Bacc()
    f32 = mybir.dt.float32
    bf16 = mybir.dt.bfloat16
    fp8 = mybir.dt.float8e3
    xg_ext = nc.declare_dram_parameter("xg", [P, B_tot, D_FEAT], fp8, isOutput=False)
    recip_ext = nc.declare_dram_parameter("recip", [P, NCHUNK], f32, isOutput=False)
    ident_ext = nc.declare_dram_parameter("ident", [P, P], fp8, isOutput=False)
    out_ext = nc.declare_dram_parameter("out", [P, NCHUNK, D_FEAT], bf16, isOutput=True)

    recip_sb = nc.alloc_sbuf_tensor("recip_sb", [P, NCHUNK], f32)
    ident_sb = nc.alloc_sbuf_tensor("ident_sb", [P, P], fp8)
    xg = nc.alloc_sbuf_tensor("xg_sb", [P, B_tot, D_FEAT], fp8)
    outst = nc.alloc_sbuf_tensor("outst", [P, NCHUNK, D_FEAT], bf16)
    dve_tmp = nc.alloc_sbuf_tensor("dve_tmp", [P, DVE_TMPS, D_FEAT], f32)
    ps = nc.alloc_psum_tensor("ps", [P, PS_BUFS, 512], f32)

    # stream chunk j -> slice index
    slice_of_chunk = np.zeros(NCHUNK, np.int64)
    for s, (c0, c1) in enumerate(slices):
        slice_of_chunk[c0:c1] = s

    # per-engine chunk sequences and buffer-reuse predecessors
    pe_chunks = [j for j in range(NCHUNK) if eng[j] == 0]
    dve_chunks = [j for j in range(NCHUNK) if eng[j] == 1]

    with (
        nc.Block() as block,
        nc.semaphore("sem_id") as sem_id,
        nc.semaphore("sem_rc") as sem_rc,
        nc.semaphore("sem_x0") as sem_x0,
        nc.semaphore("sem_x1") as sem_x1,
        nc.semaphore("sem_x2") as sem_x2,
        nc.semaphore("sem_x3") as sem_x3,
        nc.semaphore("sem_x4") as sem_x4,
        nc.semaphore("sem_x5") as sem_x5,
        nc.semaphore("sem_x6") as sem_x6,
        nc.semaphore("sem_x7") as sem_x7,
        nc.semaphore("sem_x8") as sem_x8,
        nc.semaphore("sem_x9") as sem_x9,
        nc.semaphore("sem_pe") as sem_pe,
        nc.semaphore("sem_dve") as sem_dve,
        nc.semaphore("sem_div") as sem_div,
        nc.semaphore("sem_out") as sem_out,
    ):
        sem_x = [sem_x0, sem_x1, sem_x2, sem_x3, sem_x4, sem_x5, sem_x6,
                 sem_x7, sem_x8, sem_x9][:nsl]

        @block.sync
        def _(sync):
            for s, (c0, c1) in enumerate(slices):
                b0, b1 = int(S2[c0]), int(S2[c1])
                sync.dma_start(
                    out=xg[:, b0:b1, :], in_=xg_ext[:, b0:b1, :]
                ).then_inc(sem_x[s], 16)
            sync.wait_ge(sem_out, 16 * len(outg))

        @block.tensor
        def _(pe):
            pe.wait_ge(sem_id, 16)
            last_s = -1
            for i, j in enumerate(pe_chunks):
                s = int(slice_of_chunk[j])
                if s > last_s:
                    pe.wait_ge(sem_x[s], 16)
                    last_s = s
                if i >= PS_BUFS:
                    pe.wait_ge(sem_div, pe_chunks[i - PS_BUFS] + 1)
                t2 = int(T2s[j])
                for k in range(t2):
                    mm = pe.matmul(
                        ps[:, i % PS_BUFS, 0:D_FEAT],
                        lhsT=ident_sb[:],
                        rhs=xg[:, int(S2[j]) + k, :],
                        start=(k == 0),
                        stop=(k == t2 - 1),
                    )
                    if k == t2 - 1:
                        mm.then_inc(sem_pe, 1)

        @block.vector
        def _(vec):
            last_s = -1
            for i, j in enumerate(dve_chunks):
                s = int(slice_of_chunk[j])
                if s > last_s:
                    vec.wait_ge(sem_x[s], 16)
                    last_s = s
                if i >= DVE_TMPS:
                    vec.wait_ge(sem_div, dve_chunks[i - DVE_TMPS] + 1)
                b0 = int(S2[j])
                vec.tensor_reduce(
                    out=dve_tmp[:, i % DVE_TMPS, :],
                    in_=xg[:, b0 : b0 + int(T2s[j]), :].rearrange("p t f -> p f t"),
                    axis=mybir.AxisListType.X,
                    op=mybir.AluOpType.add,
                ).then_inc(sem_dve, 1)

        @block.scalar
        def _(act):
            act.dma_start(out=ident_sb[:], in_=ident_ext[:]).then_inc(sem_id, 16)
            act.dma_start(out=recip_sb[:], in_=recip_ext[:]).then_inc(sem_rc, 16)
            act.wait_ge(sem_rc, 16)
            n_pe = n_dve = 0
            for j in range(NCHUNK):
                if eng[j] == 0:
                    i = n_pe
                    n_pe += 1
                    act.wait_ge(sem_pe, i + 1)
                    src = ps[:, i % PS_BUFS, 0:D_FEAT]
                else:
                    i = n_dve
                    n_dve += 1
                    act.wait_ge(sem_dve, i + 1)
                    src = dve_tmp[:, i % DVE_TMPS, :]
                act.activation(
                    out=outst[:, j, :],
                    in_=src,
                    func=mybir.ActivationFunctionType.Copy,
                    scale=recip_sb[:, j : j + 1],
                ).then_inc(sem_div, 1)

        @block.gpsimd
        def _(gp):
            for g, (c0, c1) in enumerate(outg):
                gp.wait_ge(sem_div, c1)
                gp.dma_start(
                    out=out_ext[:, c0:c1, :], in_=outst[:, c0:c1, :]
                ).then_inc(sem_out, 16)

    nc.finalize()
    return nc


def _get_built(x, edge_index):
    cfg, in_maps = _preprocess(x, edge_index)
    nc = _build(cfg)
    return cfg, in_maps, nc


def _postprocess(cfg, outs):
    """outs: list per core of [P, NCHUNK, D_FEAT] arrays -> [N_NODES, D] f32."""
    stream, orders = cfg["stream"], cfg["orders"]
    full = np.empty((N_NODES, D_FEAT), np.float32)
    rank_src = (stream[:, None] * P + np.arange(P)[None, :]).ravel()
    for ci in range(NCORES):
        o = np.asarray(outs[ci]).astype(np.float32)  # [P, NCHUNK, D]
        by_rank = np.empty((NCHUNK * P, D_FEAT), np.float32)
        by_rank[rank_src] = o.transpose(1, 0, 2).reshape(-1, D_FEAT)
        full[ci * SPAN + orders[ci]] = by_rank[NPAD:]
    return full


def kernel(x, edge_index):
    from concourse.bass_utils import run_bass_kernel_spmd

    cfg, in_maps, nc = _get_built(np.asarray(x), np.asarray(edge_index))
    res = run_bass_kernel_spmd(nc, in_maps, core_ids=list(range(NCORES)))
    return _postprocess(cfg, [res.results[i]["out"] for i in range(NCORES)])
